# revision 8
# baseline (speedup 1.0000x reference)
"""Trainium2 Bass kernel for nn_RandomDropout (B=8192, S=2048, int32 ids).

Semantics (training mode of the module):
  - rows with odd batch index and n_tokens >= 10 get ONE random valid
    position dropped (argmin of jax.random.uniform(key 42) over the valid
    prefix), then the row is compacted left (shift-by-one from the drop
    position, zero appended at the end of the valid region).
  - all other rows pass through unchanged.

Device strategy (pure data parallel over 8 NeuronCores):
  - Only odd rows can ever change -> only the 4096 odd rows are processed
    on device (512 rows per core); even rows are copied host-side.
  - The random matrix r = uniform(key42, (8192, 2048)) is input-independent,
    so its per-row prefix-minimum "record" positions are precomputed on the
    host (max 18 records/row, padded to K=32) and shipped as a tiny constant.
    On device: drop = max{p in records : p < n_tokens}; n_tokens is a
    Sign+accumulate pass on the scalar engine; the compaction is a
    predicated left-shift-by-one on the vector engine (mask = iota >= thresh).
"""

import os
import sys

import numpy as np

_TRN_REPO = "/opt/trn_rl_repo"
if os.path.isdir(_TRN_REPO) and _TRN_REPO not in sys.path:  # pragma: no cover
    sys.path.insert(0, _TRN_REPO)

B, S = 8192, 2048
N_CORES = 8
MIN_TOKENS = 10
K = 32  # padded record-position count (measured max is 18)
ROWS_PER_CORE = (B // 2) // N_CORES  # 512 odd rows per core
TILES_PER_CORE = ROWS_PER_CORE // 128  # 4 tiles of [128, 2048]

_cache = {}


def _records_odd() -> np.ndarray:
    """[4096, K] int32: prefix-min record positions of r's odd rows, pad=S."""
    if "rec" in _cache:
        return _cache["rec"]
    import jax

    with jax.default_device(jax.devices("cpu")[0]):
        key = jax.random.key(42)
        r = np.asarray(jax.random.uniform(key, (B, S)))
    r_odd = r[1::2]
    pm = np.minimum.accumulate(r_odd, axis=1)
    is_rec = np.empty(r_odd.shape, dtype=bool)
    is_rec[:, 0] = True
    is_rec[:, 1:] = r_odd[:, 1:] < pm[:, :-1]
    assert int(is_rec.sum(1).max()) <= K
    rec = np.full((B // 2, K), S, np.int32)
    cc = np.cumsum(is_rec, axis=1) - 1
    rows, cols = np.nonzero(is_rec)
    rec[rows, cc[rows, cols]] = cols
    _cache["rec"] = rec
    return rec


def _build_program():
    """Build the single-core Bass program (SPMD across 8 cores)."""
    import concourse.bacc as bacc
    import concourse.mybir as mybir
    import concourse.tile as tile

    i32 = mybir.dt.int32
    f32 = mybir.dt.float32
    Alu = mybir.AluOpType

    # Bacc (not plain Bass): its compile() splits multi-sem waits into the
    # event-semaphore form the TRN2 sequencers require (<=1 wait per inst).
    nc = bacc.Bacc(
        "TRN2",
        target_bir_lowering=False,
        debug=False,
        enable_asserts=False,
        num_devices=N_CORES,
    )
    x_d = nc.dram_tensor("x", [ROWS_PER_CORE, S], i32, kind="ExternalInput").ap()
    # records, interleaved so tile t / partition p reads row t*128+p:
    # host layout [128, TILES_PER_CORE * K]
    rec_d = nc.dram_tensor(
        "rec", [128, TILES_PER_CORE * K], i32, kind="ExternalInput"
    ).ap()
    y_d = nc.dram_tensor("y", [ROWS_PER_CORE, S], i32, kind="ExternalOutput").ap()

    with tile.TileContext(nc) as tc:
        with (
            tc.tile_pool(name="const", bufs=1) as const_pool,
            tc.tile_pool(name="xin", bufs=4) as xin_pool,
            tc.tile_pool(name="yout", bufs=4) as yout_pool,
            tc.tile_pool(name="mask", bufs=4) as mask_pool,
            tc.tile_pool(name="scr", bufs=4) as scr_pool,
            tc.tile_pool(name="small", bufs=8) as small_pool,
        ):
            # constants: records (one DMA), iota row index [128, S]
            rec_sb = const_pool.tile([128, TILES_PER_CORE * K], i32, tag="rec")
            nc.gpsimd.dma_start(rec_sb[:], rec_d[:])
            # bounce through DVE once so later DVE readers of the records
            # need no cross-engine waits (the STT opcode has no spare
            # sync-wait slots in codegen)
            recv = const_pool.tile([128, TILES_PER_CORE * K], i32, tag="recv")
            nc.vector.tensor_copy(recv[:], rec_sb[:])
            ones = const_pool.tile([128, S], i32, tag="ones")
            nc.vector.memset(ones[:], 1)
            iota = const_pool.tile([128, S], i32, tag="iota")
            nc.vector.tensor_tensor_scan(
                iota[:], ones[:], ones[:], -1.0, Alu.add, Alu.bypass
            )
            c_s = const_pool.tile([128, 1], f32, tag="c_s")
            nc.vector.memset(c_s[:], float(S))

            for t in range(TILES_PER_CORE):
                rows = slice(t * 128, (t + 1) * 128)
                xt = xin_pool.tile([128, S + 1], i32)
                nc.sync.dma_start(xt[:, 0:S], x_d[rows, :])
                nc.vector.memset(xt[:, S : S + 1], 0)

                # n_tokens = sum(sign(x)) on the scalar engine (ids >= 0)
                scr = scr_pool.tile([128, S], i32)
                nt = small_pool.tile([128, 1], f32, tag="nt")
                nc.scalar.activation(
                    scr[:],
                    xt[:, 0:S],
                    mybir.ActivationFunctionType.Sign,
                    accum_out=nt[:],
                )

                # drop = max(rec * (rec < n_tokens))  over K record slots.
                # ntv: DVE-side copy of nt so the STT below carries no waits.
                ntv = small_pool.tile([128, 1], f32, tag="ntv")
                nc.vector.tensor_copy(ntv[:], nt[:])
                rt = recv[:, t * K : (t + 1) * K]
                tmp = small_pool.tile([128, K], i32, tag="tmp")
                nc.vector.scalar_tensor_tensor(
                    tmp[:], rt, ntv[:], rt, Alu.is_lt, Alu.mult
                )
                drop = small_pool.tile([128, 1], f32, tag="drop")
                nc.vector.tensor_reduce(
                    drop[:], tmp[:], mybir.AxisListType.X, Alu.max
                )

                # thresh = drop where n_tokens >= MIN_TOKENS else S, i.e.
                # thr = S + g10 * (drop - S) with g10 = (x[:, 9] > 0).
                # (g10's TensorScalar also absorbs the x-DMA wait for DVE so
                # the wait-slot-less CopyPredicated below carries none.)
                g10 = small_pool.tile([128, 1], f32, tag="g10")
                nc.vector.tensor_scalar(
                    g10[:],
                    xt[:, MIN_TOKENS - 1 : MIN_TOKENS],
                    0.0,
                    None,
                    Alu.is_gt,
                    Alu.bypass,
                )
                t1 = small_pool.tile([128, 1], f32, tag="t1")
                nc.vector.tensor_scalar(
                    t1[:], drop[:], float(S), None, Alu.subtract, Alu.bypass
                )
                thr = small_pool.tile([128, 1], f32, tag="thr")
                nc.vector.scalar_tensor_tensor(
                    thr[:], g10[:], t1[:], c_s[:], Alu.mult, Alu.add
                )

                # mask = (iota >= thresh)
                mk = mask_pool.tile([128, S], i32)
                nc.vector.tensor_scalar(
                    mk[:], iota[:], thr[:], None, Alu.is_ge, Alu.bypass
                )

                # y = x;  y[j] = x[j+1] where j >= thresh
                yt = yout_pool.tile([128, S], i32)
                nc.scalar.copy(yt[:], xt[:, 0:S])
                # tiny DVE read of yt absorbs the ACT-copy wait
                obs = small_pool.tile([128, 1], i32, tag="obs")
                nc.vector.tensor_copy(obs[:], yt[:, 0:1])
                nc.vector.copy_predicated(yt[:], mk[:], xt[:, 1 : S + 1])
                nc.sync.dma_start(y_d[rows, :], yt[:])

    nc.finalize()
    return nc


def _get_program():
    if "nc" not in _cache:
        _cache["nc"] = _build_program()
    return _cache["nc"]


def _shard_inputs(input_ids: np.ndarray):
    """Per-core in_maps: odd rows of the core's contiguous 1024-row block."""
    rec = _records_odd()
    odd = input_ids[1::2]  # [4096, S]; row k <-> global row 2k+1
    in_maps = []
    for c in range(N_CORES):
        sl = slice(c * ROWS_PER_CORE, (c + 1) * ROWS_PER_CORE)
        rec_c = (
            rec[sl]
            .reshape(TILES_PER_CORE, 128, K)
            .transpose(1, 0, 2)
            .reshape(128, TILES_PER_CORE * K)
        )
        in_maps.append(
            {
                "x": np.ascontiguousarray(odd[sl]),
                "rec": np.ascontiguousarray(rec_c),
            }
        )
    return in_maps


def _run(input_ids: np.ndarray, trace: bool = False):
    from concourse.bass_utils import run_bass_kernel_spmd

    nc = _get_program()
    in_maps = _shard_inputs(input_ids)
    return run_bass_kernel_spmd(nc, in_maps, list(range(N_CORES)), trace=trace)


def kernel(input_ids: np.ndarray) -> np.ndarray:
    input_ids = np.ascontiguousarray(np.asarray(input_ids, dtype=np.int32))
    assert input_ids.shape == (B, S)
    res = _run(input_ids)
    out = input_ids.copy()
    out[1::2] = np.concatenate([m["y"] for m in res.results], axis=0)
    return out


# revision 9
# speedup vs baseline: 1.0105x; 1.0105x over previous
"""Trainium2 Bass kernel for nn_RandomDropout (B=8192, S=2048, int32 ids).

Semantics (training mode of the module):
  - rows with odd batch index and n_tokens >= 10 get ONE random valid
    position dropped (argmin of jax.random.uniform(key 42) over the valid
    prefix), then the row is compacted left (shift-by-one from the drop
    position, zero appended at the end of the valid region).
  - all other rows pass through unchanged.

Device strategy (pure data parallel over 8 NeuronCores):
  - Only odd rows can ever change -> only the 4096 odd rows are processed
    on device (512 rows per core); even rows are copied host-side.
  - Token ids are < 32000, so rows ship to the device as int16 (halves
    HBM traffic and doubles DVE throughput).
  - The random matrix r = uniform(key42, (8192, 2048)) is input-independent,
    so its per-row prefix-minimum "record" positions are precomputed on the
    host (max 18 records/row, padded to K=32) and shipped as a tiny constant.
    On device: drop = max{p in records : p < n_tokens}; n_tokens is a
    Sign+accumulate pass on the scalar engine; the compaction is an in-place
    predicated left-shift-by-one on the vector engine (mask = iota >= thresh).
"""

import os
import sys

import numpy as np

_TRN_REPO = "/opt/trn_rl_repo"
if os.path.isdir(_TRN_REPO) and _TRN_REPO not in sys.path:  # pragma: no cover
    sys.path.insert(0, _TRN_REPO)

B, S = 8192, 2048
N_CORES = 8
MIN_TOKENS = 10
K = 32  # padded record-position count (measured max is 18)
ROWS_PER_CORE = (B // 2) // N_CORES  # 512 odd rows per core
TILES_PER_CORE = ROWS_PER_CORE // 128  # 4 tiles of [128, 2048]

_cache = {}


def _records_odd() -> np.ndarray:
    """[4096, K] int32: prefix-min record positions of r's odd rows, pad=S."""
    if "rec" in _cache:
        return _cache["rec"]
    import jax

    with jax.default_device(jax.devices("cpu")[0]):
        key = jax.random.key(42)
        r = np.asarray(jax.random.uniform(key, (B, S)))
    r_odd = r[1::2]
    pm = np.minimum.accumulate(r_odd, axis=1)
    is_rec = np.empty(r_odd.shape, dtype=bool)
    is_rec[:, 0] = True
    is_rec[:, 1:] = r_odd[:, 1:] < pm[:, :-1]
    assert int(is_rec.sum(1).max()) <= K
    rec = np.full((B // 2, K), S, np.int32)
    cc = np.cumsum(is_rec, axis=1) - 1
    rows, cols = np.nonzero(is_rec)
    rec[rows, cc[rows, cols]] = cols
    _cache["rec"] = rec
    return rec


def _build_program():
    """Build the single-core Bass program (SPMD across 8 cores)."""
    import concourse.bacc as bacc
    import concourse.mybir as mybir
    import concourse.tile as tile

    i16 = mybir.dt.int16
    i32 = mybir.dt.int32
    f32 = mybir.dt.float32
    Alu = mybir.AluOpType

    # Bacc (not plain Bass): its compile() splits multi-sem waits into the
    # event-semaphore form the TRN2 sequencers require (<=1 wait per inst).
    nc = bacc.Bacc(
        "TRN2",
        target_bir_lowering=False,
        debug=False,
        enable_asserts=False,
        num_devices=N_CORES,
    )
    x_d = nc.dram_tensor("x", [ROWS_PER_CORE, S], i16, kind="ExternalInput").ap()
    # records, interleaved so tile t / partition p reads row t*128+p:
    # host layout [128, TILES_PER_CORE * K]
    rec_d = nc.dram_tensor(
        "rec", [128, TILES_PER_CORE * K], i32, kind="ExternalInput"
    ).ap()
    iota_d = nc.dram_tensor("iota", [128, S], i16, kind="ExternalInput").ap()
    y_d = nc.dram_tensor("y", [ROWS_PER_CORE, S], i16, kind="ExternalOutput").ap()

    with tile.TileContext(nc) as tc:
        with (
            tc.tile_pool(name="const", bufs=1) as const_pool,
            tc.tile_pool(name="xin", bufs=4) as xin_pool,
            tc.tile_pool(name="mask", bufs=4) as mask_pool,
            tc.tile_pool(name="scr", bufs=4) as scr_pool,
            tc.tile_pool(name="small", bufs=8) as small_pool,
        ):
            rec_sb = const_pool.tile([128, TILES_PER_CORE * K], i32, tag="rec")
            nc.sync.dma_start(rec_sb[:], rec_d[:])
            iota = const_pool.tile([128, S], i16, tag="iota")
            nc.sync.dma_start(iota[:], iota_d[:])

            for t in range(TILES_PER_CORE):
                rows = slice(t * 128, (t + 1) * 128)
                xt = xin_pool.tile([128, S + 1], i16)
                nc.sync.dma_start(xt[:, 0:S], x_d[rows, :])
                nc.vector.memset(xt[:, S : S + 1], 0)

                # n_tokens = sum(sign(x)) on the scalar engine (ids >= 0)
                scr = scr_pool.tile([128, S], i16)
                nt = small_pool.tile([128, 1], f32, tag="nt")
                nc.scalar.activation(
                    scr[:],
                    xt[:, 0:S],
                    mybir.ActivationFunctionType.Sign,
                    accum_out=nt[:],
                )

                # drop = max(rec * (rec < n_tokens))  over K record slots
                rt = rec_sb[:, t * K : (t + 1) * K]
                tmp = small_pool.tile([128, K], i32, tag="tmp")
                nc.vector.scalar_tensor_tensor(
                    tmp[:], rt, nt[:], rt, Alu.is_lt, Alu.mult
                )
                drop = small_pool.tile([128, 1], f32, tag="drop")
                nc.vector.tensor_reduce(
                    drop[:], tmp[:], mybir.AxisListType.X, Alu.max
                )

                # thresh = drop where n_tokens >= MIN_TOKENS else S:
                # z = (x[:,9] == 0) * S ; thr = max(z, drop)
                z = small_pool.tile([128, 1], f32, tag="z")
                nc.vector.tensor_scalar(
                    z[:],
                    xt[:, MIN_TOKENS - 1 : MIN_TOKENS],
                    0.0,
                    float(S),
                    Alu.is_equal,
                    Alu.mult,
                )
                thr = small_pool.tile([128, 1], f32, tag="thr")
                nc.vector.scalar_tensor_tensor(
                    thr[:], z[:], drop[:], z[:], Alu.max, Alu.bypass
                )

                # mask = (iota >= thresh); in-place predicated left shift
                mk = mask_pool.tile([128, S], i16)
                nc.vector.tensor_scalar(
                    mk[:], iota[:], thr[:], None, Alu.is_ge, Alu.bypass
                )
                nc.vector.copy_predicated(xt[:, 0:S], mk[:], xt[:, 1 : S + 1])
                nc.sync.dma_start(y_d[rows, :], xt[:, 0:S])

    nc.finalize()
    return nc


def _get_program():
    if "nc" not in _cache:
        _cache["nc"] = _build_program()
    return _cache["nc"]


def _shard_inputs(input_ids: np.ndarray):
    """Per-core in_maps: odd rows of the core's contiguous 1024-row block."""
    rec = _records_odd()
    odd16 = input_ids[1::2].astype(np.int16)  # ids < 32000 fit int16
    iota = np.broadcast_to(
        np.arange(S, dtype=np.int16), (128, S)
    ).copy()
    in_maps = []
    for c in range(N_CORES):
        sl = slice(c * ROWS_PER_CORE, (c + 1) * ROWS_PER_CORE)
        rec_c = (
            rec[sl]
            .reshape(TILES_PER_CORE, 128, K)
            .transpose(1, 0, 2)
            .reshape(128, TILES_PER_CORE * K)
        )
        in_maps.append(
            {
                "x": np.ascontiguousarray(odd16[sl]),
                "rec": np.ascontiguousarray(rec_c),
                "iota": iota,
            }
        )
    return in_maps


def _run(input_ids: np.ndarray, trace: bool = False):
    from concourse.bass_utils import run_bass_kernel_spmd

    nc = _get_program()
    in_maps = _shard_inputs(input_ids)
    return run_bass_kernel_spmd(nc, in_maps, list(range(N_CORES)), trace=trace)


def kernel(input_ids: np.ndarray) -> np.ndarray:
    input_ids = np.ascontiguousarray(np.asarray(input_ids, dtype=np.int32))
    assert input_ids.shape == (B, S)
    res = _run(input_ids)
    out = input_ids.copy()
    out[1::2] = np.concatenate([m["y"] for m in res.results], axis=0)
    return out
